# revision 38
# baseline (speedup 1.0000x reference)
"""Trainium2 Bass kernel for KeypointSelector:
conv3x3(384->128, pad 1) + bias + ReLU -> conv1x1(128->1) + bias + sigmoid.

Input  dino_features: (32, 64, 64, 384) f32
Output (32, 64, 64, 1) f32

Strategy: pure data parallel over batch, 4 images per core on 8 cores.
Conv3x3 runs on the PE array in fp8e4m3 with MatmulPerfMode.DoubleRow:
each matmul contracts TWO 128-deep k-tiles (cin-chunk x tap pairs) at
0.5 cycles per output column -- 4x the bf16 column rate. The 27 k-tiles
(3 cin chunks x 9 taps) are padded to 28 and paired so that both slices
of every pair live at a constant column offset (delta) inside one flat
[128, 3*4356] SBUF image tile; all deltas exceed the 512-column moving
tile so no access pattern overlaps.

fp8 scaling: x is pre-scaled by 16 and w1 by 256 host-side (w1 values
~1e-2 would land in the fp8 subnormal range unscaled); the 1/4096
descale is folded into the ReLU activation's scale argument. The 1x1
conv + sigmoid stay in bf16/f32. Measured end-to-end rel err ~6e-3.

Weight-reuse groups: tiles are processed in groups of 3 with the pair
loop outermost (k-outer, tile-inner), so each DoubleRow LDWEIGHTS serves
3 matmuls and stays hidden under the previous matmul on hardware.
"""

import ml_dtypes
import numpy as np

import concourse.bass as bass
import concourse.tile as tile
from concourse import bacc, mybir
from concourse.ap import AP
from concourse.bass_utils import run_bass_kernel_spmd

BF16 = ml_dtypes.bfloat16
E4M3 = ml_dtypes.float8_e4m3  # == mybir.dt.np(float8e4)

# Geometry
B, H, W, CIN, CHID = 32, 64, 64, 384, 128
NCORES = 8
BLOC = B // NCORES  # 4 images per core
HP, WP = H + 2, W + 2  # 66x66 padded grid
NPIX = HP * WP  # 4356 padded pixels per image
TS = 512  # matmul free-dim tile (one PSUM bank of fp32)
START = WP + 1  # padded idx of first valid output pixel (1,1) = 67
END = H * WP + W + 1  # 4289: one past padded idx of pixel (63,63)
NT = -(-(END - START) // TS)  # 9 tiles per image (last one partial)
TILE_N = [min(TS, END - START - t * TS) for t in range(NT)]  # [512]*8 + [126]
NCHUNK = CIN // 128  # 3 cin chunks
GROUPS = [[0, 1, 2], [3, 4, 5], [6, 7, 8]]  # weight-reuse tile groups
OUTW = NT * TS  # padded flat output row (4608); only first 4222 cols valid
OUTV = END - START  # 4222 valid flat output cols

# fp8 scaling
SX, SW = 16.0, 256.0
DESCALE = 1.0 / (SX * SW)

# k-tile pairing for DoubleRow: 27 (chunk, tap) k-tiles + 1 zero slot -> 14
# pairs. Tap t=(dy+1)*3+(dx+1) reads at padded-col offset OFF[t]; chunk c
# lives at flat offset c*NPIX. Pairs are chosen cross-chunk so every rhs
# delta is >= NPIX-134 > 512 (no overlapping access patterns).
OFF = [-WP - 1, -WP, -WP + 1, -1, 0, 1, WP - 1, WP, WP + 1]
PAIRS = (
    [(0, t, 1, t) for t in range(5)]
    + [(0, 5 + j, 2, j) for j in range(4)]
    + [(1, 5 + j, 2, 4 + j) for j in range(4)]
    + [(None, None, 2, 8)]  # slot A holds zero weights
)
NPAIR = len(PAIRS)  # 14
BASEA, DELTA = [], []
for cA, tA, cB, tB in PAIRS:
    if cA is None:
        # zero-weight A slice: point at valid chunk-1 data, delta to chunk 2
        BASEA.append(1 * NPIX + OFF[tB])
        DELTA.append(NPIX)
    else:
        BASEA.append(cA * NPIX + OFF[tA])
        DELTA.append((cB - cA) * NPIX + OFF[tB] - OFF[tA])
assert all(d > TS for d in DELTA)

_CACHED = {}


def _build_bass(reps=1):
    nc = bacc.Bacc("TRN2", target_bir_lowering=False)

    f32 = mybir.dt.float32
    bf16 = mybir.dt.bfloat16
    fp8 = mybir.dt.float8e4
    DR = mybir.MatmulPerfMode.DoubleRow

    x = nc.dram_tensor("x", [BLOC, NCHUNK, 128, NPIX], fp8, kind="ExternalInput")
    w1 = nc.dram_tensor("w1", [128, NPAIR, 2, CHID], fp8, kind="ExternalInput")
    b1 = nc.dram_tensor("b1", [CHID, 1], f32, kind="ExternalInput")
    w2 = nc.dram_tensor("w2", [CHID, 1], bf16, kind="ExternalInput")
    b2 = nc.dram_tensor("b2", [97, 1], f32, kind="ExternalInput")
    y = nc.dram_tensor("y", [BLOC, OUTW], f32, kind="ExternalOutput")

    with tile.TileContext(nc) as tc:
        with (
            tc.tile_pool(name="consts", bufs=1) as consts,
            tc.tile_pool(name="xin", bufs=2) as xin,
            tc.tile_pool(name="hbuf", bufs=4) as hbuf,
            tc.tile_pool(name="obuf", bufs=2) as obuf,
            tc.tile_pool(name="ps1", bufs=2, space="PSUM") as ps1,
            tc.tile_pool(name="ps2", bufs=2, space="PSUM") as ps2,
        ):
            # Cold-start critical path: only w1 (needed by the first
            # Ldweights) is loaded now, via the ACT queue so the image DMAs
            # lead the SP queue. The small consts (first needed ~9us in) are
            # deferred until after image 0's DMAs are enqueued.
            w1_s = consts.tile([128, NPAIR, 2, CHID], fp8)
            nc.sync.dma_start(out=w1_s[:], in_=w1[:])
            # Zeros tile: second operand of the DVE relu (max against 0),
            # and fodder for the p-state priming matmuls below.
            z_s = consts.tile([CHID, TS], bf16)
            nc.scalar.memzero(z_s[:])
            b1_s = consts.tile([CHID, 1], f32)
            w2_s = consts.tile([CHID, 1], bf16)
            b2_s = consts.tile([97, 1], f32)  # b2 replicated host-side

            def load_small_consts():
                nc.scalar.dma_start(out=b1_s, in_=b1[:])
                nc.scalar.dma_start(out=w2_s, in_=w2[:])
                nc.scalar.dma_start(out=b2_s, in_=b2[:])

            # Prime the PE p-state ramp (full clock needs ~3us of continuous
            # busy) with throwaway matmuls on the zeros tile while the first
            # image's DMAs are in flight.
            prime = ps2.tile([CHID, TS], f32, name="prime", tag="p2x")
            for _ in range(7):
                nc.tensor.matmul(out=prime[:, :TS], lhsT=z_s[:, 0:CHID],
                                 rhs=z_s[:, 0:TS], start=True, stop=True)

            # One-group software pipeline: group g's ReLUs/1x1s are emitted
            # after group g+1's conv matmuls so the PE never queues behind
            # the ACT engine. `pend` carries [(h_s, n, tile_idx)] and img.
            #
            # The three 1x1 outputs of a group land at PSUM partitions
            # {0,32,64} of one bank (PE tile_position column offsets), so one
            # 3-lane sigmoid serves the whole group and one strided DMA
            # writes it out.
            pend = None

            def flush(pend):
                tiles, img = pend
                p2x = ps2.tile([97, TS], f32, name="p2x", tag="p2x")
                for g, (h_s, n, t) in enumerate(tiles):
                    nc.tensor.matmul(out=p2x[32 * g:32 * g + 1, :n],
                                     lhsT=w2_s[:], rhs=h_s[:, :n],
                                     start=True, stop=True)
                out_g = obuf.tile([97, TS], f32, name="og", tag="og")
                ostr = list(out_g.ap)[0][0]
                for g, (h_s, n, t) in enumerate(tiles):
                    nc.scalar.activation(
                        out=out_g[32 * g:32 * g + 1, :n],
                        in_=p2x[32 * g:32 * g + 1, :n],
                        func=mybir.ActivationFunctionType.Sigmoid,
                        bias=b2_s[32 * g:32 * g + 1], scale=1.0,
                    )
                full = [(g, t) for g, (h, n, t) in enumerate(tiles)
                        if n == TS]
                part = [(g, n, t) for g, (h, n, t) in enumerate(tiles)
                        if n != TS]
                if full:
                    # DMA APs may stride partitions (engine APs may not):
                    # one strided DMA writes the group's full tiles.
                    m = len(full)
                    t0 = full[0][1]
                    nc.sync.dma_start(
                        out=y[img, t0 * TS:(t0 + m) * TS],
                        in_=AP(out_g.tensor, out_g.offset,
                               [[32 * ostr, m], [1, TS]]))
                for g, n, t in part:
                    nc.sync.dma_start(
                        out=y[img, t * TS:t * TS + n],
                        in_=out_g[32 * g:32 * g + 1, :n])

            iters = [ii for _ in range(reps) for ii in range(BLOC)]
            for idx, i in enumerate(iters):
                last_image = idx == len(iters) - 1
                # Flat fp8 image tile: 3 cin chunks side by side; one DMA
                # per chunk (HWDGE descriptor-gen is ~630ns per DMA, so
                # fewer, larger DMAs win), chunk 1 on the gpsimd SWDGE.
                xf = xin.tile([128, NCHUNK * NPIX], fp8, tag="x")
                pstride = list(xf.ap)[0][0]
                # All chunks on the SP queue: the serial DMA pipe delivers
                # in emission order. Image 0 is split into halves ordered
                # [c0a, c1a, c2a, c0b, c1b, c2b] so the first conv pair
                # (chunks 0+1) starts after ~2.8us of transfers instead of
                # waiting for whole chunks.
                segs = [(0, NPIX // 2), (NPIX // 2, NPIX)] if idx == 0 \
                    else [(0, NPIX)]
                for a, b in segs:
                    for ch in range(NCHUNK):
                        nc.sync.dma_start(
                            out=xf[:, ch * NPIX + a:ch * NPIX + b],
                            in_=x[i, ch, :, a:b])
                if idx == 0:
                    load_small_consts()

                def conv(ptile, t, k):
                    n = TILE_N[t]
                    s0 = START + t * TS
                    rhs = AP(xf.tensor, xf.offset + BASEA[k] + s0,
                             [[pstride, 128], [DELTA[k], 2], [1, n]])
                    nc.tensor.matmul(
                        out=ptile[:, :n],
                        lhsT=w1_s[:, k],
                        rhs=rhs,
                        start=(k == 0),
                        stop=(k == NPAIR - 1),
                        perf_mode=DR,
                    )

                def relu(ptile, t):
                    # h' = max(p + 4096*b1, 0) = 4096*relu(p/4096 + b1) on
                    # the otherwise-idle DVE; the 4096 descale is folded
                    # into w2 host-side. Keeps the ACT engine sigmoid-only.
                    n = TILE_N[t]
                    h_s = hbuf.tile([CHID, TS], bf16, tag=f"h{t % 3}",
                                    name=f"h_{t % 3}")
                    nc.vector.scalar_tensor_tensor(
                        out=h_s[:, :n], in0=ptile[:, :n], scalar=b1_s[:],
                        in1=z_s[:, :n], op0=mybir.AluOpType.add,
                        op1=mybir.AluOpType.max,
                    )
                    return (h_s, n, t)

                for gi, grp in enumerate(GROUPS):
                    if not (last_image and gi == len(GROUPS) - 1):
                        ptiles = [ps1.tile([CHID, TS], f32, tag=f"p{g % 3}",
                                           name=f"p1_{g % 3}")
                                  for g in grp]
                        # k-outer / tile-inner: one weight load per pair
                        # serves the whole group.
                        for k in range(NPAIR):
                            for g, t in enumerate(grp):
                                conv(ptiles[g], t, k)
                        if pend is not None:
                            flush(pend)
                        tiles = [relu(ptiles[g], t)
                                 for g, t in enumerate(grp)]
                        pend = (tiles, i)
                    else:
                        # Final group of the whole kernel: tile-serial conv
                        # emission with per-tile relu->1x1->sigmoid->DMA so
                        # each tile's chain overlaps the next tile's convs
                        # and the drain tail is one small chain.
                        p2x = ps2.tile([97, TS], f32, name="p2xt",
                                       tag="p2x")
                        out_g = obuf.tile([97, TS], f32, name="ogt",
                                          tag="og")
                        hprev = None

                        def tail_tile(g, h_s, n, t):
                            nc.tensor.matmul(
                                out=p2x[32 * g:32 * g + 1, :n],
                                lhsT=w2_s[:], rhs=h_s[:, :n],
                                start=True, stop=True)
                            nc.scalar.activation(
                                out=out_g[32 * g:32 * g + 1, :n],
                                in_=p2x[32 * g:32 * g + 1, :n],
                                func=mybir.ActivationFunctionType.Sigmoid,
                                bias=b2_s[32 * g:32 * g + 1], scale=1.0,
                            )
                            nc.sync.dma_start(
                                out=y[i, t * TS:t * TS + n],
                                in_=out_g[32 * g:32 * g + 1, :n])

                        for g, t in enumerate(grp):
                            pt = ps1.tile([CHID, TS], f32, tag=f"p{g % 3}",
                                          name=f"pt_{g % 3}")
                            for k in range(NPAIR):
                                conv(pt, t, k)
                            if g == 0 and pend is not None:
                                flush(pend)
                            if hprev is not None:
                                tail_tile(hprev[0], *relu(*hprev[1:]))
                            hprev = (g, pt, t)
                        tail_tile(hprev[0], *relu(*hprev[1:]))
    nc.compile()
    return nc


def _prep_inputs(dino_features, W1, b1, W2, b2):
    xp = np.zeros((B, HP, WP, CIN), dtype=np.float32)
    xp[:, 1:H + 1, 1:W + 1, :] = dino_features * SX
    # -> [B, chunk, cin_in_chunk, padded_pixel]
    xq = np.ascontiguousarray(
        xp.transpose(0, 3, 1, 2).reshape(B, NCHUNK, 128, NPIX)).astype(E4M3)

    # W1 (3,3,384,128) (ky,kx,ci,co) -> [chunk, cin128, tap, cout], then pair
    wq = (np.asarray(W1) * SW).astype(E4M3)
    wr = wq.transpose(2, 0, 1, 3).reshape(NCHUNK, 128, 9, CHID)
    w1p = np.zeros((128, NPAIR, 2, CHID), dtype=E4M3)
    for k, (cA, tA, cB, tB) in enumerate(PAIRS):
        if cA is not None:
            w1p[:, k, 0, :] = wr[cA, :, tA, :]
        w1p[:, k, 1, :] = wr[cB, :, tB, :]

    # DVE relu computes h' = max(p + 4096*b1, 0) = 4096*h, so scale b1 up
    # and w2 down by the fp8 descale factor.
    b1h = np.ascontiguousarray(
        (b1.reshape(CHID, 1) / DESCALE).astype(np.float32))
    w2h = np.ascontiguousarray(
        (W2.reshape(CHID, 1) * DESCALE).astype(BF16))
    b2h = np.full((97, 1), np.float32(b2.reshape(())), dtype=np.float32)

    in_maps = []
    for c in range(NCORES):
        in_maps.append({
            "x": np.ascontiguousarray(xq[c * BLOC:(c + 1) * BLOC]),
            "w1": w1p, "b1": b1h, "w2": w2h, "b2": b2h,
        })
    return in_maps


def kernel(dino_features, W1, b1, W2, b2, _trace=False, _trace_kwargs=None):
    if "nc" not in _CACHED:
        _CACHED["nc"] = _build_bass()
    nc = _CACHED["nc"]
    in_maps = _prep_inputs(dino_features, W1, b1, W2, b2)
    res = run_bass_kernel_spmd(nc, in_maps, core_ids=list(range(NCORES)),
                               trace=_trace, **(_trace_kwargs or {}))
    _CACHED["last_results"] = res
    out = np.concatenate([res.results[c]["y"] for c in range(NCORES)], axis=0)
    # Flat col 66*h + w (w<64) -> pixel (h, w); cols >= OUTV are padding.
    out = out[:, :H * WP].reshape(B, H, WP)[:, :, :W]
    return np.ascontiguousarray(out).reshape(B, H, W, 1).astype(np.float32)


# revision 57
# speedup vs baseline: 748.8572x; 748.8572x over previous
"""Trainium2 Bass kernel for KeypointSelector:
conv3x3(384->128, pad 1) + bias + ReLU -> conv1x1(128->1) + bias + sigmoid.

Input  dino_features: (32, 64, 64, 384) f32
Output (32, 64, 64, 1) f32

Strategy: pure data parallel over batch, 4 images per core on 8 cores.
Conv3x3 runs on the PE array in fp8e4m3 with MatmulPerfMode.DoubleRow:
each matmul contracts TWO 128-deep k-tiles (cin-chunk x tap pairs) at
0.5 cycles per output column -- 4x the bf16 column rate. The 27 k-tiles
(3 cin chunks x 9 taps) are padded to 28 and paired so that both slices
of every pair live at a constant column offset (delta) inside one flat
[128, 3*4356] SBUF image tile; all deltas exceed the 512-column moving
tile so no access pattern overlaps.

fp8 scaling: x is pre-scaled by 16 and w1 by 256 host-side (w1 values
~1e-2 would land in the fp8 subnormal range unscaled); the 1/4096
descale is folded into the ReLU activation's scale argument. The 1x1
conv + sigmoid stay in bf16/f32. Measured end-to-end rel err ~6e-3.

Weight-reuse groups: tiles are processed in groups of 3 with the pair
loop outermost (k-outer, tile-inner), so each DoubleRow LDWEIGHTS serves
3 matmuls and stays hidden under the previous matmul on hardware.
"""

import ml_dtypes
import numpy as np

import concourse.bass as bass
import concourse.tile as tile
from concourse import bacc, mybir
from concourse.ap import AP
from concourse.bass_utils import run_bass_kernel_spmd

BF16 = ml_dtypes.bfloat16
E4M3 = ml_dtypes.float8_e4m3  # == mybir.dt.np(float8e4)

# Geometry
B, H, W, CIN, CHID = 32, 64, 64, 384, 128
NCORES = 8
BLOC = B // NCORES  # 4 images per core
HP, WP = H + 2, W + 2  # 66x66 padded grid
NPIX = HP * WP  # 4356 padded pixels per image
TS = 512  # matmul free-dim tile (one PSUM bank of fp32)
START = WP + 1  # padded idx of first valid output pixel (1,1) = 67
END = H * WP + W + 1  # 4289: one past padded idx of pixel (63,63)
NT = -(-(END - START) // TS)  # 9 tiles per image (last one partial)
TILE_N = [min(TS, END - START - t * TS) for t in range(NT)]  # [512]*8 + [126]
NCHUNK = CIN // 128  # 3 cin chunks
GROUPS = [[0, 1], [2, 3], [4, 5], [6, 7], [8]]  # weight-reuse tile groups
OUTW = NT * TS  # padded flat output row (4608); only first 4222 cols valid
OUTV = END - START  # 4222 valid flat output cols

# fp8 scaling
SX, SW = 16.0, 256.0
DESCALE = 1.0 / (SX * SW)
SW2 = 16.0  # 1x1 weight pre-scale for fp8 range
W2DESCALE = 1.0 / SW2

# k-tile pairing for DoubleRow: 27 (chunk, tap) k-tiles + 1 bias slot -> 14
# pairs. Tap t=(dy+1)*3+(dx+1) reads at padded-col offset OFF[t]; chunk c
# lives at flat offset c*NPIX. Pairs are chosen cross-chunk so every rhs
# delta is >= NPIX-134 > 512 (no overlapping access patterns).
#
# The image tile carries an all-ones region at flat offset 3*NPIX, aligned
# so that ones-col = 3*NPIX + (s0 - START) + j for output column j. Pair 13
# pairs the leftover k-tile (chunk2, tap8) with the ones region whose
# weights hold 4096*b1/128 per row -- the conv bias folded into the matmul.
OFF = [-WP - 1, -WP, -WP + 1, -1, 0, 1, WP - 1, WP, WP + 1]
ONES_W = 4608  # ones-region columns (covers s0 - START + n for every tile)
XFW = NCHUNK * NPIX + ONES_W  # flat image tile width
PAIRS = (
    [(0, t, 1, t) for t in range(5)]
    + [(0, 5 + j, 2, j) for j in range(4)]
    + [(1, 5 + j, 2, 4 + j) for j in range(4)]
    + [(2, 8, None, None)]  # slot B reads the ones region (bias fold)
)
NPAIR = len(PAIRS)  # 14
BASEA, DELTA = [], []
for cA, tA, cB, tB in PAIRS:
    if cB is None:
        BASEA.append(cA * NPIX + OFF[tA])
        DELTA.append((NCHUNK * NPIX - START) - (cA * NPIX + OFF[tA]))
    else:
        BASEA.append(cA * NPIX + OFF[tA])
        DELTA.append((cB - cA) * NPIX + OFF[tB] - OFF[tA])
assert all(d > TS for d in DELTA)

_CACHED = {}


def _build_bass(reps=1):
    nc = bacc.Bacc("TRN2", target_bir_lowering=False)

    f32 = mybir.dt.float32
    bf16 = mybir.dt.bfloat16
    fp8 = mybir.dt.float8e4
    DR = mybir.MatmulPerfMode.DoubleRow

    x = nc.dram_tensor("x", [BLOC, NCHUNK, 128, NPIX], fp8, kind="ExternalInput")
    w1 = nc.dram_tensor("w1", [128, NPAIR, 2, CHID], fp8, kind="ExternalInput")
    ones = nc.dram_tensor("ones", [128, ONES_W], fp8, kind="ExternalInput")
    # 1x1 weights for the DoubleRow trick: dual-fp8 LDWEIGHTS requires >=16
    # stationary columns, so w2*16 sits in (slot 0, col 0) of a [128, 2, 16]
    # zero block. The rhs duplicates h via a stride-0 k-tile dim (slot-1
    # weights are zero), giving 0.5 cycles/column without repacking h.
    w2 = nc.dram_tensor("w2", [128, 2, 16], fp8, kind="ExternalInput")
    b2 = nc.dram_tensor("b2", [97, 1], f32, kind="ExternalInput")
    y = nc.dram_tensor("y", [BLOC, OUTW], f32, kind="ExternalOutput")

    with tile.TileContext(nc) as tc:
        with (
            tc.tile_pool(name="consts", bufs=1) as consts,
            tc.tile_pool(name="xin", bufs=2) as xin,
            tc.tile_pool(name="hbuf", bufs=4) as hbuf,
            tc.tile_pool(name="obuf", bufs=2) as obuf,
            tc.tile_pool(name="ps1", bufs=2, space="PSUM") as ps1,
            tc.tile_pool(name="ps2", bufs=2, space="PSUM") as ps2,
        ):
            # Cold-start critical path: only w1 (needed by the first
            # Ldweights) is loaded now, via the ACT queue so the image DMAs
            # lead the SP queue. The small consts (first needed ~9us in) are
            # deferred until after image 0's DMAs are enqueued.
            w1_s = consts.tile([128, NPAIR, 2, CHID], fp8)
            nc.sync.dma_start(out=w1_s[:], in_=w1[:])
            # Zeros tile: second operand of the DVE relu (max against 0),
            # and fodder for the p-state priming matmuls below.
            z_s = consts.tile([CHID, TS], bf16)
            nc.scalar.memzero(z_s[:])
            w2_s = consts.tile([128, 2, 16], fp8)
            b2_s = consts.tile([97, 1], f32)  # b2 replicated host-side

            def load_small_consts():
                nc.scalar.dma_start(out=w2_s, in_=w2[:])
                nc.scalar.dma_start(out=b2_s, in_=b2[:])

            # Prime the PE p-state ramp (full clock needs ~3us of continuous
            # busy) with throwaway matmuls on the zeros tile while the first
            # image's DMAs are in flight.
            prime = ps2.tile([CHID, TS], f32, name="prime", tag="p2_0")
            for _ in range(7):
                nc.tensor.matmul(out=prime[:, :TS], lhsT=z_s[:, 0:CHID],
                                 rhs=z_s[:, 0:TS], start=True, stop=True)

            # One-group software pipeline: group g's ReLUs/1x1s are emitted
            # after group g+1's conv matmuls so the PE never queues behind
            # the ACT engine. `pend` carries [(h_s, n, tile_idx)] and img.
            #
            # Each 1x1 runs as a DoubleRow fp8 matmul at 0.5 cycles/column:
            # dual-fp8 LDWEIGHTS needs >=16 stationary cols and dst
            # partition 0, so each 1x1 gets its own PSUM bank writing rows
            # 0..15 (row 0 real, rest zero filler). The sigmoid then shifts
            # the result to out_g row 32g so one strided DMA per group
            # writes y.
            pend = None

            def flush(pend):
                hh, spans, img = pend
                hstr = list(hh.ap)[0][0]
                p2s = []
                for g, n, t in spans:
                    p2 = ps2.tile([16, TS], f32, name=f"p2_{g}",
                                  tag=f"p2_{g}")
                    rhs = AP(hh.tensor, hh.offset + g * TS,
                             [[hstr, 128], [0, 2], [1, n]])
                    nc.tensor.matmul(
                        out=p2[0:16, :n], lhsT=w2_s[:],
                        rhs=rhs, start=True, stop=True, perf_mode=DR)
                    p2s.append(p2)
                out_g = obuf.tile([97, TS], f32, name="og", tag="og")
                ostr = list(out_g.ap)[0][0]
                for (g, n, t), p2 in zip(spans, p2s):
                    nc.scalar.activation(
                        out=out_g[32 * g:32 * g + 1, :n],
                        in_=p2[0:1, :n],
                        func=mybir.ActivationFunctionType.Sigmoid,
                        bias=b2_s[0:1], scale=W2DESCALE,
                    )
                full = [(g, t) for g, n, t in spans if n == TS]
                part = [(g, n, t) for g, n, t in spans if n != TS]
                if full:
                    # DMA APs may stride partitions (engine APs may not):
                    # one strided DMA writes the group's full tiles.
                    m = len(full)
                    t0 = full[0][1]
                    nc.sync.dma_start(
                        out=y[img, t0 * TS:(t0 + m) * TS],
                        in_=AP(out_g.tensor, out_g.offset,
                               [[32 * ostr, m], [1, TS]]))
                for g, n, t in part:
                    nc.sync.dma_start(
                        out=y[img, t * TS:t * TS + n],
                        in_=out_g[32 * g:32 * g + 1, :n])

            iters = [ii for _ in range(reps) for ii in range(BLOC)]
            for idx, i in enumerate(iters):
                last_image = idx == len(iters) - 1
                # Flat fp8 image tile: 3 cin chunks side by side; one DMA
                # per chunk (HWDGE descriptor-gen is ~630ns per DMA, so
                # fewer, larger DMAs win), chunk 1 on the gpsimd SWDGE.
                xf = xin.tile([128, XFW], fp8, tag="x")
                pstride = list(xf.ap)[0][0]
                # All chunks on the SP queue: the serial DMA pipe delivers
                # in emission order. Image 0 is split into halves ordered
                # [c0a, c1a, c2a, c0b, c1b, c2b] so the first conv pair
                # (chunks 0+1) starts after ~2.8us of transfers instead of
                # waiting for whole chunks.
                segs = [(0, NPIX // 2), (NPIX // 2, NPIX)] if idx == 0 \
                    else [(0, NPIX)]
                for a, b in segs:
                    for ch in range(NCHUNK):
                        nc.sync.dma_start(
                            out=xf[:, ch * NPIX + a:ch * NPIX + b],
                            in_=x[i, ch, :, a:b])
                if idx < 2:
                    # Fill this buffer's ones region (pair 13's slot-B data,
                    # the conv-bias fold). The xin pool has 2 buffers; later
                    # images reuse them with the ones region untouched.
                    nc.sync.dma_start(
                        out=xf[:, NCHUNK * NPIX:XFW], in_=ones[:])
                if idx == 0:
                    load_small_consts()

                def conv(ptile, t, k):
                    n = TILE_N[t]
                    s0 = START + t * TS
                    rhs = AP(xf.tensor, xf.offset + BASEA[k] + s0,
                             [[pstride, 128], [DELTA[k], 2], [1, n]])
                    nc.tensor.matmul(
                        out=ptile[:, :n],
                        lhsT=w1_s[:, k],
                        rhs=rhs,
                        start=(k == 0),
                        stop=(k == NPAIR - 1),
                        perf_mode=DR,
                    )

                def relu(ptile, t, hh, g):
                    # h = max(p/4096, 0) = relu(p/4096 + b1) on the
                    # otherwise-idle DVE (b1 was folded into conv pair 13),
                    # written as fp8 for the DoubleRow 1x1.
                    n = TILE_N[t]
                    nc.vector.scalar_tensor_tensor(
                        out=hh[:, g * TS:g * TS + n], in0=ptile[:, :n],
                        scalar=DESCALE, in1=z_s[:, :n],
                        op0=mybir.AluOpType.mult, op1=mybir.AluOpType.max,
                    )
                    return (g, n, t)

                for gi, grp in enumerate(GROUPS):
                    ptiles = [ps1.tile([CHID, TS], f32, tag=f"p{g % 2}",
                                       name=f"p1_{g % 2}")
                              for g in grp]
                    # k-outer / tile-inner: one weight load per pair serves
                    # the whole group.
                    for k in range(NPAIR):
                        for g, t in enumerate(grp):
                            conv(ptiles[g], t, k)
                    if pend is not None:
                        flush(pend)
                    hh = hbuf.tile([CHID, 2 * TS], fp8, tag="hh",
                                   name="hh")
                    spans = [relu(ptiles[g], t, hh, g)
                             for g, t in enumerate(grp)]
                    pend = (hh, spans, i)
            flush(pend)
    nc.compile()
    return nc


def _prep_inputs(dino_features, W1, b1, W2, b2):
    xp = np.zeros((B, HP, WP, CIN), dtype=np.float32)
    xp[:, 1:H + 1, 1:W + 1, :] = dino_features * SX
    # -> [B, chunk, cin_in_chunk, padded_pixel]
    xq = np.ascontiguousarray(
        xp.transpose(0, 3, 1, 2).reshape(B, NCHUNK, 128, NPIX)).astype(E4M3)

    # W1 (3,3,384,128) (ky,kx,ci,co) -> [chunk, cin128, tap, cout], then pair
    wq = (np.asarray(W1) * SW).astype(E4M3)
    wr = wq.transpose(2, 0, 1, 3).reshape(NCHUNK, 128, 9, CHID)
    w1p = np.zeros((128, NPAIR, 2, CHID), dtype=E4M3)
    for k, (cA, tA, cB, tB) in enumerate(PAIRS):
        w1p[:, k, 0, :] = wr[cA, :, tA, :]
        if cB is not None:
            w1p[:, k, 1, :] = wr[cB, :, tB, :]
        else:
            # Bias fold: slot B multiplies the all-ones region; each of the
            # 128 rows contributes (4096*b1)/128 = 32*b1 to PSUM.
            w1p[:, k, 1, :] = np.broadcast_to(
                (32.0 * np.asarray(b1).reshape(1, CHID)).astype(E4M3),
                (128, CHID))

    w2p = np.zeros((128, 2, 16), dtype=E4M3)
    w2p[:, 0, 0] = (np.asarray(W2).reshape(CHID) * SW2).astype(E4M3)
    b2h = np.full((97, 1), np.float32(b2.reshape(())), dtype=np.float32)
    onesh = np.ones((128, ONES_W), dtype=E4M3)

    in_maps = []
    for c in range(NCORES):
        in_maps.append({
            "x": np.ascontiguousarray(xq[c * BLOC:(c + 1) * BLOC]),
            "w1": w1p, "ones": onesh, "w2": w2p, "b2": b2h,
        })
    return in_maps


def kernel(dino_features, W1, b1, W2, b2, _trace=False, _trace_kwargs=None):
    if "nc" not in _CACHED:
        _CACHED["nc"] = _build_bass()
    nc = _CACHED["nc"]
    in_maps = _prep_inputs(dino_features, W1, b1, W2, b2)
    res = run_bass_kernel_spmd(nc, in_maps, core_ids=list(range(NCORES)),
                               trace=_trace, **(_trace_kwargs or {}))
    _CACHED["last_results"] = res
    out = np.concatenate([res.results[c]["y"] for c in range(NCORES)], axis=0)
    # Flat col 66*h + w (w<64) -> pixel (h, w); cols >= OUTV are padding.
    out = out[:, :H * WP].reshape(B, H, WP)[:, :, :W]
    return np.ascontiguousarray(out).reshape(B, H, W, 1).astype(np.float32)
